# revision 1
# baseline (speedup 1.0000x reference)
"""Batched attention (no 1/sqrt(d) scaling) for Trainium2, 8 NeuronCores.

Problem: q,k,v [128, 1024, 64] fp32 ->
    out = softmax(q @ k^T, axis=-1) @ v   per batch.

Sharding: batch dim 128 split as 16 batches per core (data parallel, no
communication).

Per-core algorithm (per batch b):
  1. Load q,k as [128, 8, 64] tiles; PE-transpose into qT,kT [64, 1024]
     (d on partitions).
  2. For each t-tile (128 rows of k): scoresT[t, s] = kT_tile^T @ qT in
     PSUM [128, 1024]; exp via ACT into SBUF expT (no max subtraction:
     |scores| <= ~45, safe in fp32 and exp is ~2ulp accurate).
  3. Accumulate out'^T [65, 1024] += [v_tile | 1]^T @ expT over t-tiles.
     Row 64 is the softmax denominator (ones-column trick).
  4. PE-transpose out'^T per s-tile -> [128, 65]; multiply cols 0:64 by
     reciprocal of col 64; DMA out.

This keeps softmax weights in the [t, s] layout the AV matmul needs, so no
[128x128] attention transposes are required.
"""

import os
import sys
from contextlib import ExitStack

sys.path.insert(0, "/opt/trn_rl_repo")

import numpy as np

import concourse.bass as bass
import concourse.tile as tile
from concourse import mybir
from concourse.bass_utils import run_bass_kernel_spmd
from concourse.masks import make_identity

# ---------------------------------------------------------------------------
# Workaround: this walrus build allows only ONE semaphore wait per
# instruction (TPB_EVENTS has a single wait slot).  The stock Tile epilogue
# attaches every outstanding sem wait to a single SP Drain, which fails
# codegen with "Too many sync wait commands".  Split the waits across one
# Drain per semaphore instead (SP executes them sequentially, semantics are
# identical).
# ---------------------------------------------------------------------------
import bass_rust
from concourse.vector_clock import ScopedClock


def _split_wait_drain_and_barrier(self, tick_clock, wait_clock):
    nc = self.nc
    drain_inst = nc.sync.drain()
    wait_clock.add_sem_waits(
        drain_inst.ins, ScopedClock({None: tick_clock.global_clock})
    )
    ins = drain_inst.ins
    si = ins.sync_info
    if si is not None and si.on_wait and len(si.on_wait) > 1:
        waits = list(si.on_wait)
        si.on_wait = waits[:1]
        for w in waits[1:]:
            extra = nc.sync.drain()
            extra_ins = extra.ins
            if extra_ins.sync_info is None:
                extra_ins.sync_info = bass_rust.SyncInfo(on_wait=[w], on_update=[])
            else:
                extra_ins.sync_info.on_wait = [w]

    nc.all_engine_barrier()
    assert self.sems is not None
    popped = nc._tile_sem_poison_stack.pop()
    assert popped is self._sem_poison
    nc.clear_and_free_semaphores(list(self.sems.allocated().values()))
    nc.all_engine_barrier()


tile.TileContext._drain_and_barrier = _split_wait_drain_and_barrier


def _legalize_single_wait(nc):
    """Rewrite every instruction carrying N>1 sem waits into N-1 single-wait
    NoOps (same engine, inserted just before it) + the instruction keeping one
    wait.  Same-engine execution is in-order, so semantics are preserved."""
    fn = nc.m.functions[0]
    for blk in fn.blocks:
        insts = blk.instructions
        if not any(
            i.sync_info is not None and i.sync_info.on_wait and len(i.sync_info.on_wait) > 1
            for i in insts
        ):
            continue
        out = []
        for inst in insts:
            si = inst.sync_info
            if si is not None and si.on_wait and len(si.on_wait) > 1:
                waits = list(si.on_wait)
                for w in waits[:-1]:
                    out.append(
                        mybir.InstNoOp(
                            name=nc.get_next_instruction_name(),
                            engine=inst.engine,
                            sync_info=mybir.SyncInfo(on_wait=[w], on_update=[]),
                            bass_nofuse=True,
                        )
                    )
                si.on_wait = waits[-1:]
            out.append(inst)
        blk.instructions = out


# ---------------------------------------------------------------------------

N_CORES = 8
B, S, D = 128, 1024, 64
B_LOC = B // N_CORES  # batches per core
NT = S // 128  # 128-row tiles per sequence

# Matmul input dtype modes: "f32r" streams fp32 at 1 cycle/row (reduced
# precision), "f32" is exact but 4 cycles/row, "split3" (MM1 only) computes
# q@k^T as qh.kh + qh.kl + ql.kh with hi/lo f32r splits -- near-fp32 accuracy
# at 3 cycles/row.
MM1_MODE = os.environ.get("ATTN_MM1", "f32r")
MM2_MODE = os.environ.get("ATTN_MM2", "f32r")

F32 = mybir.dt.float32
F32R = mybir.dt.float32r

# The BIR verifier requires operands consumed by an fp32r matmul to be
# *written* as fp32r (producer rounds on write), so the tiles feeding the
# matmuls carry the dtype rather than a bitcast at the matmul site.
MM1_DT = F32 if MM1_MODE == "f32" else F32R
MM2_DT = F32R if MM2_MODE == "f32r" else F32


def _attention_body(tc, o, q, k, v, reps=1, variant="full"):
    nc = tc.nc
    with ExitStack() as ctx:
        singles = ctx.enter_context(tc.tile_pool(name="singles", bufs=1))
        ident = singles.tile([128, 128], F32)
        make_identity(nc, ident)
        ones8 = singles.tile([128, NT], F32)
        nc.vector.memset(ones8, 1.0)

        qk_pool = ctx.enter_context(tc.tile_pool(name="qk", bufs=int(os.environ.get("ATTN_QK_BUFS", "2"))))
        qkt_pool = ctx.enter_context(tc.tile_pool(name="qkt", bufs=2))
        v_pool = ctx.enter_context(tc.tile_pool(name="vp", bufs=2))
        exp_pool = ctx.enter_context(tc.tile_pool(name="expp", bufs=int(os.environ.get("ATTN_EXP_BUFS", "3"))))
        outT_pool = ctx.enter_context(tc.tile_pool(name="outTp", bufs=2))
        out_pool = ctx.enter_context(tc.tile_pool(name="outp", bufs=2))
        r_pool = ctx.enter_context(tc.tile_pool(name="rp", bufs=4))
        # PSUM budget (8 banks): misc 2x[128,512] = 2, scores 2x[128,1024] = 4,
        # accumulator 1x[65,1024] = 2.
        ps_misc = ctx.enter_context(tc.tile_pool(name="ps_misc", bufs=2, space="PSUM"))
        ps_sc = ctx.enter_context(tc.tile_pool(name="ps_sc", bufs=2, space="PSUM"))
        ps_acc = ctx.enter_context(tc.tile_pool(name="ps_acc", bufs=1, space="PSUM"))

        def emit_tail(ps_o, ob):
            # Transpose the accumulator back per s-tile and normalize by the
            # denominator row (row 64 of ps_o).
            outT = outT_pool.tile([D + 1, S], F32, tag="outT")
            if os.environ.get("ATTN_OUTT_COPY", "act") == "act":
                nc.scalar.copy(outT, ps_o)
            else:
                nc.vector.tensor_copy(outT, ps_o)
            ou = out_pool.tile([128, NT, D], F32, tag="ou")
            for sidx in range(NT):
                ps_tr2 = ps_misc.tile([128, 512], F32, tag="tr")
                nc.tensor.transpose(
                    ps_tr2[:, 0 : D + 1],
                    outT[:, bass.ts(sidx, 128)],
                    ident[0 : D + 1, 0 : D + 1],
                )
                rec = r_pool.tile([128, 1], F32, tag="rec")
                nc.vector.reciprocal(rec, ps_tr2[:, D : D + 1])
                nc.vector.tensor_scalar_mul(ou[:, sidx, :], ps_tr2[:, 0:D], rec)
            if variant != "nodma":
                nc.sync.dma_start(out=ob, in_=ou)

        if variant == "nodma":
            qn0 = singles.tile([128, NT, D], F32)
            nc.sync.dma_start(out=qn0, in_=q[0].rearrange("(n p) d -> p n d", p=128))
            kn0 = singles.tile([128, NT, D], F32)
            nc.sync.dma_start(out=kn0, in_=k[0].rearrange("(n p) d -> p n d", p=128))
            va0 = singles.tile([128, NT, D + 1], MM2_DT)
            nc.sync.dma_start(
                out=va0[:, :, 0:D],
                in_=v[0].rearrange("(n p) d -> p n d", p=128).bitcast(MM2_DT),
            )
            nc.vector.tensor_copy(va0[:, :, D], ones8)

        def emit_all():
            pending_tail = None  # (ps_o, ob) of the previous batch
            for b in range(B_LOC):
                qb = q[b].rearrange("(n p) d -> p n d", p=128)
                kb = k[b].rearrange("(n p) d -> p n d", p=128)
                vb = v[b].rearrange("(n p) d -> p n d", p=128)
                ob = o[b].rearrange("(n p) d -> p n d", p=128)

                if variant == "nodma":
                    qn, kn, va = qn0, kn0, va0
                else:
                    qn = qk_pool.tile([128, NT, D], F32, tag="qn")
                    nc.sync.dma_start(out=qn, in_=qb)
                    kn = qk_pool.tile([128, NT, D], F32, tag="kn")
                    nc.sync.dma_start(out=kn, in_=kb)
                    va = v_pool.tile([128, NT, D + 1], MM2_DT, tag="va")
                    nc.sync.dma_start(out=va[:, :, 0:D], in_=vb.bitcast(MM2_DT))
                    nc.vector.tensor_copy(va[:, :, D], ones8)
                if variant == "dmaonly":
                    nc.sync.dma_start(out=ob, in_=qn.bitcast(F32))
                    continue

                # qT/kT [64, 1024]: PE transposes of the 8 [128, 64] subtiles,
                # staged 4-at-a-time through one PSUM bank.  k-subtile 0 first so
                # MM1(0) unblocks as early as possible.
                qT = qkt_pool.tile([D, S], MM1_DT, tag="qT")
                kT = qkt_pool.tile([D, S], MM1_DT, tag="kT")
                if MM1_MODE == "split3":
                    qTlo = qkt_pool.tile([D, S], F32R, tag="qTlo")
                    kTlo = qkt_pool.tile([D, S], F32R, tag="kTlo")
                    los = {id(qT): qTlo, id(kT): kTlo}
                for srcT, dstT in ((kn, kT), (qn, qT)):
                    for j in range(2):
                        ps_tr = ps_misc.tile([128, 512], F32, tag="tr")
                        for i in range(4):
                            n = j * 4 + i
                            nc.tensor.transpose(
                                ps_tr[0:D, bass.ts(i, 128)], srcT[:, n, :], ident
                            )
                        nc.vector.tensor_copy(dstT[:, bass.ts(j, 512)], ps_tr[0:D, :])
                        if MM1_MODE == "split3":
                            # lo = exact - hi, rounded to f32r; qh.kh+qh.kl+ql.kh
                            # then recovers ~fp32-accurate scores.
                            nc.vector.tensor_sub(
                                los[id(dstT)][:, bass.ts(j, 512)],
                                ps_tr[0:D, :],
                                dstT[:, bass.ts(j, 512)],
                            )

                # Previous batch's epilogue goes here: its PE work (8 small
                # transposes) slots between this batch's input transposes and
                # MM1s, so ACT/PE never wait on it at a batch boundary.  It also
                # releases the single ps_acc slot before MM2(0) below needs it.
                if pending_tail is not None:
                    emit_tail(*pending_tail)

                # Main loop over t-tiles: scoresT -> exp -> accumulate AV.
                ps_o = ps_acc.tile([D + 1, S], F32, tag="acc")
                for n in range(NT):
                    ps_s = ps_sc.tile([128, S], F32, tag="sc")
                    if MM1_MODE == "split3":
                        passes = [(kT, qT, True, False), (kT, qTlo, False, False),
                                  (kTlo, qT, False, True)]
                    else:
                        passes = [(kT, qT, True, True)]
                    for lhs_src, rhs_src, st, sp in passes:
                        lhsT = lhs_src[:, bass.ts(n, 128)]
                        for h in range(2):
                            nc.tensor.matmul(
                                ps_s[:, bass.ts(h, 512)],
                                lhsT,
                                rhs_src[:, bass.ts(h, 512)],
                                start=st,
                                stop=sp,
                            )
                    expT = exp_pool.tile([128, S], MM2_DT, tag="expT")
                    nc.scalar.activation(expT, ps_s, mybir.ActivationFunctionType.Exp)
                    lhsT2 = va[:, n, :]
                    for h in range(2):
                        nc.tensor.matmul(
                            ps_o[:, bass.ts(h, 512)],
                            lhsT2,
                            expT[:, bass.ts(h, 512)],
                            start=(n == 0),
                            stop=(n == NT - 1),
                        )
                pending_tail = (ps_o, ob)

            if pending_tail is not None:
                emit_tail(*pending_tail)

        if reps <= 1:
            emit_all()
        else:
            with tc.For_i(0, reps, 1):
                emit_all()


def build_nc(b_loc=B_LOC, reps=1, legalize=True, variant="full"):
    nc = bass.Bass("TRN2", target_bir_lowering=False, debug=False)
    q = nc.dram_tensor("q", [b_loc, S, D], F32, kind="ExternalInput").ap()
    k = nc.dram_tensor("k", [b_loc, S, D], F32, kind="ExternalInput").ap()
    v = nc.dram_tensor("v", [b_loc, S, D], F32, kind="ExternalInput").ap()
    o = nc.dram_tensor("out", [b_loc, S, D], F32, kind="ExternalOutput").ap()

    global B_LOC_ACTIVE
    B_LOC_ACTIVE = b_loc
    saved = globals()["B_LOC"]
    globals()["B_LOC"] = b_loc
    try:
        with tile.TileContext(nc) as tc:
            _attention_body(tc, o, q, k, v, reps=reps, variant=variant)
        if legalize:
            _legalize_single_wait(nc)
    finally:
        globals()["B_LOC"] = saved
    return nc


LAST_RESULTS = None
LAST_RUN_WALL_S = None
_NC_CACHE = {}


def kernel(q, k, v):
    import time as _time

    q = np.ascontiguousarray(np.asarray(q, dtype=np.float32))
    k = np.ascontiguousarray(np.asarray(k, dtype=np.float32))
    v = np.ascontiguousarray(np.asarray(v, dtype=np.float32))
    assert q.shape == (B, S, D), q.shape

    if "nc" not in _NC_CACHE:
        _NC_CACHE["nc"] = build_nc()
    nc = _NC_CACHE["nc"]
    in_maps = []
    for c in range(N_CORES):
        sl = slice(c * B_LOC, (c + 1) * B_LOC)
        in_maps.append({"q": q[sl], "k": k[sl], "v": v[sl]})

    t0 = _time.time()
    res = run_bass_kernel_spmd(nc, in_maps, list(range(N_CORES)))
    global LAST_RESULTS, LAST_RUN_WALL_S
    LAST_RUN_WALL_S = _time.time() - t0
    LAST_RESULTS = res
    out = np.concatenate([res.results[c]["out"] for c in range(N_CORES)], axis=0)
    return out



# revision 14
# speedup vs baseline: 1.2146x; 1.2146x over previous
"""Batched attention (no 1/sqrt(d) scaling) for Trainium2, 8 NeuronCores.

Problem: q,k,v [128, 1024, 64] fp32 ->
    out = softmax(q @ k^T, axis=-1) @ v   per batch.

Sharding: batch dim 128 split as 16 batches per core (data parallel, no
communication).

Per-core algorithm (per batch b), designed so the Activation engine (the only
engine that can do exp, at 1 elem/lane/cycle) is the sole bottleneck:

  1. Load q,k as f32r [128, 8, 64] tiles; PE-transpose (f32r: 1.5 cyc/row)
     into qT,kT [64, 1024] (d on partitions), staged through PSUM with the
     PSUM->SBUF copies split between DVE (k) and GpSimd (q).
  2. Load v, convert to bf16 va [128, 8, 65] with a ones column (col 64).
  3. For each t-tile (128 rows of k): scoresT[t, s] = kT_tile^T @ qT in PSUM
     [128, 1024] (f32r, 1 cyc/col); exp via ACT straight into bf16 expT
     (no max subtraction: |scores| <= ~45, exp fits fp32/bf16 range).
  4. MM2 with exp as the STATIONARY operand: for each s-tile,
     acc[s, 0:65] += expT[:, s-tile]^T @ va[t]  -- output lands directly in
     [s, d] layout (plus denominator in col 64), bf16 at 1 cyc/col with only
     65 moving columns.  No attention transposes, no output transposes, no
     PSUM->SBUF accumulator copy.
  5. Normalize on DVE: out[s, 0:64] = acc[s, 0:64] * recip(acc[s, 64]);
     DMA out.

Cross-batch software pipelining: batch b's last two MM2 tile-steps are
emitted after batch b+1's input transposes so the PE keeps the ACT engine
fed across batch boundaries.
"""

import os
import sys
from contextlib import ExitStack

sys.path.insert(0, "/opt/trn_rl_repo")

import numpy as np

import concourse.bass as bass
import concourse.tile as tile
from concourse import mybir
from concourse.bass_utils import run_bass_kernel_spmd
from concourse.masks import make_identity

# ---------------------------------------------------------------------------
# Workaround: this walrus build allows only ONE semaphore wait per
# instruction (TPB_EVENTS has a single wait slot).  The stock Tile epilogue
# attaches every outstanding sem wait to a single SP Drain, which fails
# codegen with "Too many sync wait commands".  Split the waits across one
# Drain per semaphore instead (SP executes them sequentially, semantics are
# identical).
# ---------------------------------------------------------------------------
import bass_rust
from concourse.vector_clock import ScopedClock


def _split_wait_drain_and_barrier(self, tick_clock, wait_clock):
    nc = self.nc
    drain_inst = nc.sync.drain()
    wait_clock.add_sem_waits(
        drain_inst.ins, ScopedClock({None: tick_clock.global_clock})
    )
    ins = drain_inst.ins
    si = ins.sync_info
    if si is not None and si.on_wait and len(si.on_wait) > 1:
        waits = list(si.on_wait)
        si.on_wait = waits[:1]
        for w in waits[1:]:
            extra = nc.sync.drain()
            extra_ins = extra.ins
            if extra_ins.sync_info is None:
                extra_ins.sync_info = bass_rust.SyncInfo(on_wait=[w], on_update=[])
            else:
                extra_ins.sync_info.on_wait = [w]

    nc.all_engine_barrier()
    assert self.sems is not None
    popped = nc._tile_sem_poison_stack.pop()
    assert popped is self._sem_poison
    nc.clear_and_free_semaphores(list(self.sems.allocated().values()))
    nc.all_engine_barrier()


tile.TileContext._drain_and_barrier = _split_wait_drain_and_barrier


def _legalize_single_wait(nc):
    """Rewrite every instruction carrying N>1 sem waits into N-1 single-wait
    NoOps (same engine, inserted just before it) + the instruction keeping one
    wait.  Same-engine execution is in-order, so semantics are preserved."""
    fn = nc.m.functions[0]
    for blk in fn.blocks:
        insts = blk.instructions
        if not any(
            i.sync_info is not None and i.sync_info.on_wait and len(i.sync_info.on_wait) > 1
            for i in insts
        ):
            continue
        out = []
        for inst in insts:
            si = inst.sync_info
            if si is not None and si.on_wait and len(si.on_wait) > 1:
                waits = list(si.on_wait)
                for w in waits[:-1]:
                    out.append(
                        mybir.InstNoOp(
                            name=nc.get_next_instruction_name(),
                            engine=inst.engine,
                            sync_info=mybir.SyncInfo(on_wait=[w], on_update=[]),
                            bass_nofuse=True,
                        )
                    )
                si.on_wait = waits[-1:]
            out.append(inst)
        blk.instructions = out


# ---------------------------------------------------------------------------

N_CORES = 8
B, S, D = 128, 1024, 64
B_LOC = B // N_CORES  # batches per core
NT = S // 128  # 128-row tiles per sequence

F32 = mybir.dt.float32
F32R = mybir.dt.float32r
BF16 = mybir.dt.bfloat16

# Transpose-path dtype: f32r transposes are 1.5 cyc/row vs 2.0 for f32.
TR_DT = F32R if os.environ.get("ATTN_TR", "f32r") == "f32r" else F32


def _attention_body(tc, o, q, k, v):
    nc = tc.nc
    with ExitStack() as ctx:
        singles = ctx.enter_context(tc.tile_pool(name="singles", bufs=1))
        # GPSIMD memset can't write f32r directly; build in f32, copy once.
        ident_f32 = singles.tile([128, 128], F32)
        make_identity(nc, ident_f32)
        ident = singles.tile([128, 128], TR_DT)
        nc.vector.tensor_copy(ident, ident_f32)

        qk_pool = ctx.enter_context(tc.tile_pool(name="qk", bufs=2))
        v_pool = ctx.enter_context(tc.tile_pool(name="vp", bufs=2))
        va_pool = ctx.enter_context(tc.tile_pool(name="vap", bufs=2))
        qkt_pool = ctx.enter_context(tc.tile_pool(name="qkt", bufs=2))
        exp_pool = ctx.enter_context(tc.tile_pool(name="expp", bufs=4))
        out_pool = ctx.enter_context(tc.tile_pool(name="outp", bufs=2))
        acc_pool = ctx.enter_context(tc.tile_pool(name="accp", bufs=2))
        r_pool = ctx.enter_context(tc.tile_pool(name="rp", bufs=4))
        # PSUM budget (8 banks): transpose staging 2x[64,512] = 2, scores
        # 2x[128,1024] = 4, output accumulator 1x[128,8,128] = 2.
        ps_tr_pool = ctx.enter_context(tc.tile_pool(name="ps_tr", bufs=2, space="PSUM"))
        ps_sc = ctx.enter_context(tc.tile_pool(name="ps_sc", bufs=2, space="PSUM"))
        ps_acc = ctx.enter_context(tc.tile_pool(name="ps_acc", bufs=1, space="PSUM"))

        def emit_mm2(ps_o, va, expT, n):
            # exp as stationary: acc[s, 0:65] += expT[:, s-tile]^T @ va[t].
            # start=True resets the ENTIRE psum bank (verified on HW), which
            # would wipe the other 3 s-slots sharing the bank -- so the bank
            # is zeroed once by DVE memset and every matmul accumulates.
            for sidx in range(NT):
                nc.tensor.matmul(
                    ps_o[:, sidx, 0 : D + 1],
                    expT[:, bass.ts(sidx, 128)],
                    va[:, n, :],
                    start=False,
                    stop=False,
                    skip_group_check=True,
                )

        def emit_norm(ps_o, ob):
            # One fast PSUM->SBUF copy releases the accumulator for the next
            # batch's MM2s; reciprocal+scale then run off the critical chain.
            accs = acc_pool.tile([128, NT, D + 1], F32, tag="accs")
            nc.vector.tensor_copy(accs, ps_o[:, :, 0 : D + 1])
            # DVE reciprocal with free size > 1 miscomputes on HW (only the
            # last element of each 4-group is right) -- keep it [128, 1].
            ou = out_pool.tile([128, NT, D], F32, tag="ou")
            for sidx in range(NT):
                rec = r_pool.tile([128, 1], F32, tag="rec")
                nc.vector.reciprocal(rec, accs[:, sidx, D : D + 1])
                nc.vector.tensor_scalar_mul(ou[:, sidx, :], accs[:, sidx, 0:D], rec)
            nc.sync.dma_start(out=ob, in_=ou)

        # pending per-batch tail work carried into the next batch's emission:
        # (ps_o, va, expT6, expT7, ob) -- MM2 for t-tiles 6,7 + normalize.
        pending = None

        for b in range(B_LOC):
            qb = q[b].rearrange("(n p) d -> p n d", p=128)
            kb = k[b].rearrange("(n p) d -> p n d", p=128)
            vb = v[b].rearrange("(n p) d -> p n d", p=128)
            ob = o[b].rearrange("(n p) d -> p n d", p=128)

            qn = qk_pool.tile([128, NT, D], TR_DT, tag="qn")
            nc.sync.dma_start(out=qn, in_=qb.bitcast(TR_DT))
            kn = qk_pool.tile([128, NT, D], TR_DT, tag="kn")
            nc.sync.dma_start(out=kn, in_=kb.bitcast(TR_DT))
            vn = v_pool.tile([128, NT, D], F32, tag="vn")
            nc.sync.dma_start(out=vn, in_=vb)

            # qT/kT [64, 1024]: PE transposes of the 8 [128, 64] subtiles,
            # staged 4-at-a-time through PSUM.  k first (MM1 needs its slice
            # stationary first).  GPSIMD cannot read PSUM, so both staging
            # copies go to DVE.
            qT = qkt_pool.tile([D, S], F32R, tag="qT")
            kT = qkt_pool.tile([D, S], F32R, tag="kT")
            for srcT, dstT, eng in ((kn, kT, nc.vector), (qn, qT, nc.vector)):
                for j in range(2):
                    ps_tr = ps_tr_pool.tile([D, 512], TR_DT, tag="tr")
                    for i in range(4):
                        n = j * 4 + i
                        nc.tensor.transpose(
                            ps_tr[:, bass.ts(i, 128)], srcT[:, n, :], ident
                        )
                    eng.tensor_copy(dstT[:, bass.ts(j, 512)], ps_tr)

            # v -> bf16 with ones column (col 64 accumulates the softmax
            # denominator through MM2).
            va = va_pool.tile([128, NT, D + 1], BF16, tag="va")
            nc.vector.tensor_copy(va[:, :, 0:D], vn)
            nc.vector.memset(va[:, :, D], 1.0)

            def emit_mm1(ps_s, n):
                lhsT = kT[:, bass.ts(n, 128)]
                for h in range(2):
                    nc.tensor.matmul(
                        ps_s[:, bass.ts(h, 512)],
                        lhsT,
                        qT[:, bass.ts(h, 512)],
                        start=True,
                        stop=True,
                    )

            # MM1(b, 0) can run during exp(b-1, 7): its score slot only needs
            # exp(b-1, 6) done.  Emit it BEFORE the previous batch's pending
            # MM2(6,7) (which wait on exp(b-1, 7)) so ACT's first exp of this
            # batch starts right after the previous batch's last one.
            ps_s0 = ps_sc.tile([128, S], F32, tag="sc")
            emit_mm1(ps_s0, 0)

            # Previous batch's tail: its MM2(6,7) PE work runs while this
            # batch's staging copies complete, and its DVE normalize overlaps
            # this batch's first MM1/exp.
            if pending is not None:
                p_ps_o, p_va, p_e6, p_e7, p_ob = pending
                emit_mm2(p_ps_o, p_va, p_e6, NT - 2)
                emit_mm2(p_ps_o, p_va, p_e7, NT - 1)
                emit_norm(p_ps_o, p_ob)

            # Main loop over t-tiles: scoresT -> exp(bf16) -> MM2 accumulate.
            ps_o = ps_acc.tile([128, NT, 128], F32, tag="acc")
            nc.vector.memset(ps_o, 0.0)
            expTs = []
            for n in range(NT):
                if n == 0:
                    ps_s = ps_s0
                else:
                    ps_s = ps_sc.tile([128, S], F32, tag="sc")
                    emit_mm1(ps_s, n)
                expT = exp_pool.tile([128, S], BF16, tag="expT")
                nc.scalar.activation(expT, ps_s, mybir.ActivationFunctionType.Exp)
                expTs.append(expT)
                # Keep PE two MM1 tiles ahead of MM2 so ACT never starves.
                if n >= 2:
                    emit_mm2(ps_o, va, expTs[n - 2], n - 2)
            pending = (ps_o, va, expTs[NT - 2], expTs[NT - 1], ob)

        p_ps_o, p_va, p_e6, p_e7, p_ob = pending
        emit_mm2(p_ps_o, p_va, p_e6, NT - 2)
        emit_mm2(p_ps_o, p_va, p_e7, NT - 1)
        emit_norm(p_ps_o, p_ob)


def build_nc(b_loc=B_LOC, legalize=True):
    nc = bass.Bass("TRN2", target_bir_lowering=False, debug=False)
    q = nc.dram_tensor("q", [b_loc, S, D], F32, kind="ExternalInput").ap()
    k = nc.dram_tensor("k", [b_loc, S, D], F32, kind="ExternalInput").ap()
    v = nc.dram_tensor("v", [b_loc, S, D], F32, kind="ExternalInput").ap()
    o = nc.dram_tensor("out", [b_loc, S, D], F32, kind="ExternalOutput").ap()

    saved = globals()["B_LOC"]
    globals()["B_LOC"] = b_loc
    try:
        with tile.TileContext(nc) as tc:
            _attention_body(tc, o, q, k, v)
        if legalize:
            _legalize_single_wait(nc)
    finally:
        globals()["B_LOC"] = saved
    return nc


LAST_RESULTS = None
LAST_RUN_WALL_S = None
_NC_CACHE = {}


def kernel(q, k, v):
    import time as _time

    q = np.ascontiguousarray(np.asarray(q, dtype=np.float32))
    k = np.ascontiguousarray(np.asarray(k, dtype=np.float32))
    v = np.ascontiguousarray(np.asarray(v, dtype=np.float32))
    assert q.shape == (B, S, D), q.shape

    if "nc" not in _NC_CACHE:
        _NC_CACHE["nc"] = build_nc()
    nc = _NC_CACHE["nc"]
    in_maps = []
    for c in range(N_CORES):
        sl = slice(c * B_LOC, (c + 1) * B_LOC)
        in_maps.append({"q": q[sl], "k": k[sl], "v": v[sl]})

    t0 = _time.time()
    res = run_bass_kernel_spmd(nc, in_maps, list(range(N_CORES)))
    global LAST_RESULTS, LAST_RUN_WALL_S
    LAST_RUN_WALL_S = _time.time() - t0
    LAST_RESULTS = res
    out = np.concatenate([res.results[c]["out"] for c in range(N_CORES)], axis=0)
    return out
